# revision 12
# baseline (speedup 1.0000x reference)
"""Trainium2 Bass kernel for the moe_routing prompt-selection module.

Math (per the reference):
    xf    = mean over tokens of x [B, F*197, ED] -> [B*F, ED]
    query = xf @ W_in^T; q = l2norm(query)          [BZ, PD]
    keys  = l2norm(prompt_values[:, 0, :])          [SIZE, PD]
    sim   = q @ keys^T                              [BZ, SIZE]
    idx   = top_k(sim, 8)  (descending)
    out   = (keys[idx] @ W_out^T) laid out [B, F*K, ED]
    ps_loss = (sum||(sim*top8mask)@keys - q||^2/BZ + sum|keys@keys^T - I|/BZ) * F

Sharding: data-parallel over batch; each of 8 cores handles 4 batches
(64 rows of BZ=512).  x is split host-side into fp16 hi+lo halves so the
big token-reduction matmul runs at full PE rate with fp32-level accuracy
(products are exact in the fp32 PSUM accumulator; the split residual is
~2^-23 relative).  The token mean is computed as S^T @ x where S is a
0/1 frame-selection matrix (exact in fp16); all other matmuls are fp32.
"""

import os

os.environ.setdefault("JAX_COMPILATION_CACHE_DIR", "/tmp/jax_kernel_cache")

import numpy as np

B, F, NT = 32, 16, 197
ED, PD, SIZE, K = 768, 256, 64, 8
NCORES = 8
BP = B // NCORES            # batches per core
BF = BP * F                 # 64 sim-rows per core
TP = BP * F * NT            # 12608 tokens per core
RK = BF * K                 # 512 output rows per core
NCHUNK = (TP + 127) // 128  # 99 (last chunk has 64 rows)
TP_PAD = NCHUNK * 128

_BUILT = None


def _build_body(tc, ctx, aps):
    import concourse.mybir as mybir
    from concourse.mybir import AluOpType as alu
    from concourse.mybir import ActivationFunctionType as act

    nc = tc.nc
    f32 = mybir.dt.float32
    f16 = mybir.dt.float16
    X = mybir.AxisListType.X

    xb, ssel, winT, woutT, pv, ident, out_mat, dn_out, kk_out = aps

    consts = ctx.enter_context(tc.tile_pool(name="consts", bufs=1))
    small = ctx.enter_context(tc.tile_pool(name="small", bufs=1))
    scratch = ctx.enter_context(tc.tile_pool(name="scratch", bufs=2))
    xpool = ctx.enter_context(tc.tile_pool(name="xpool", bufs=8))
    opool = ctx.enter_context(tc.tile_pool(name="opool", bufs=2))
    psA = ctx.enter_context(tc.tile_pool(name="psA", bufs=2, space="PSUM"))
    psS = ctx.enter_context(tc.tile_pool(name="psS", bufs=2, space="PSUM"))

    # ---- constant loads -------------------------------------------------
    st = consts.tile([128, NCHUNK, BF], f16)
    nc.sync.dma_start(st, ssel.rearrange("(c p) m -> p c m", p=128))
    winT_sb = consts.tile([128, 6, PD], f32)
    nc.sync.dma_start(winT_sb, winT.rearrange("(i p) q -> p i q", p=128))
    woutT_sb = consts.tile([128, 2, ED], f32)
    nc.sync.dma_start(woutT_sb, woutT.rearrange("(i p) e -> p i e", p=128))
    pv_sb = consts.tile([SIZE, PD], f32)
    nc.sync.dma_start(pv_sb, pv)
    id_sb = consts.tile([128, 128], f32)
    nc.sync.dma_start(id_sb, ident)
    id64 = id_sb[:SIZE, :SIZE]

    # ---- phase A: keys, keys@keys^T row-sums, KW = keys @ W_out^T ------
    kn2 = small.tile([SIZE, 1], f32)
    sqk = scratch.tile([SIZE, PD], f32, tag="sq")
    nc.scalar.activation(sqk, pv_sb, act.Square, accum_out=kn2)
    kstd = small.tile([SIZE, 1], f32)
    nc.scalar.activation(kstd, kn2, act.Sqrt)
    krs = small.tile([SIZE, 1], f32)
    nc.vector.reciprocal(krs, kstd)
    keys_sb = small.tile([SIZE, PD], f32)
    nc.scalar.activation(keys_sb, pv_sb, act.Copy, scale=krs)

    kT = []
    for i in range(2):
        tp = psS.tile([128, SIZE], f32, tag="tp")
        nc.tensor.transpose(tp, keys_sb[:, i * 128:(i + 1) * 128], id64)
        kTs = small.tile([128, SIZE], f32, name=f"kT{i}")
        nc.vector.tensor_copy(kTs, tp)
        kT.append(kTs)

    kkps = psS.tile([SIZE, SIZE], f32, tag="mm")
    nc.tensor.matmul(kkps, kT[0], kT[0], start=True, stop=False)
    nc.tensor.matmul(kkps, kT[1], kT[1], start=False, stop=True)
    kkd = scratch.tile([SIZE, SIZE], f32, tag="kkd")
    nc.vector.tensor_tensor(kkd, kkps, id64, op=alu.subtract)
    kkrow_sb = small.tile([SIZE, 1], f32)
    nc.vector.tensor_reduce(kkrow_sb, kkd, axis=X, op=alu.add,
                            apply_absolute_value=True)
    nc.sync.dma_start(kk_out, kkrow_sb)

    kwps = psA.tile([SIZE, ED], f32, tag="acc")
    for i in range(2):
        for n0, nw in ((0, 512), (512, 256)):
            nc.tensor.matmul(kwps[:, n0:n0 + nw], kT[i],
                             woutT_sb[:, i, n0:n0 + nw],
                             start=(i == 0), stop=(i == 1))
    kw_sb = small.tile([SIZE, ED], f32)
    nc.vector.tensor_copy(kw_sb, kwps)

    # ---- phase B: token-reduction stream  xf = S^T @ (x_hi + x_lo) -----
    xf_ps = psA.tile([BF, ED], f32, tag="acc")
    n_mm = 0
    total_mm = NCHUNK * 4
    for c in range(NCHUNK):
        kk = 128 if c < NCHUNK - 1 else TP - 128 * (NCHUNK - 1)
        xt = xpool.tile([128, 2, ED], f16, tag="xt")
        nc.sync.dma_start(xt[:kk], xb[c * 128:c * 128 + kk])
        for h in range(2):
            for n0, nw in ((0, 512), (512, 256)):
                nc.tensor.matmul(xf_ps[:, n0:n0 + nw], st[:kk, c],
                                 xt[:kk, h, n0:n0 + nw],
                                 start=(n_mm < 2), stop=(n_mm >= total_mm - 2))
                n_mm += 1
    xf_sb = small.tile([BF, ED], f32)
    nc.vector.tensor_copy(xf_sb, xf_ps)

    # ---- phase C: query / sim / top-k / outputs ------------------------
    xfT = []
    for i in range(6):
        tp = psS.tile([128, SIZE], f32, tag="tp")
        nc.tensor.transpose(tp, xf_sb[:, i * 128:(i + 1) * 128], id64)
        xfTs = small.tile([128, BF], f32, name=f"xfT{i}")
        nc.vector.tensor_copy(xfTs, tp)
        xfT.append(xfTs)

    q_ps = psS.tile([BF, PD], f32, tag="mm")
    for i in range(6):
        nc.tensor.matmul(q_ps, xfT[i], winT_sb[:, i],
                         start=(i == 0), stop=(i == 5))

    qn2 = small.tile([BF, 1], f32)
    sqq = scratch.tile([BF, PD], f32, tag="sq")
    nc.scalar.activation(sqq, q_ps, act.Square, accum_out=qn2)
    qstd = small.tile([BF, 1], f32)
    nc.scalar.activation(qstd, qn2, act.Sqrt)
    qrs = small.tile([BF, 1], f32)
    nc.vector.reciprocal(qrs, qstd)
    q_sb = small.tile([BF, PD], f32)
    nc.scalar.activation(q_sb, q_ps, act.Copy, scale=qrs)

    qT = []
    for i in range(2):
        tp = psS.tile([128, SIZE], f32, tag="tp")
        nc.tensor.transpose(tp, q_sb[:, i * 128:(i + 1) * 128], id64)
        qTs = small.tile([128, BF], f32, name=f"qT{i}")
        nc.vector.tensor_copy(qTs, tp)
        qT.append(qTs)

    sim_ps = psS.tile([BF, SIZE], f32, tag="mm")
    nc.tensor.matmul(sim_ps, qT[0], kT[0], start=True, stop=False)
    nc.tensor.matmul(sim_ps, qT[1], kT[1], start=False, stop=True)
    sim_sb = small.tile([BF, SIZE], f32)
    nc.vector.tensor_copy(sim_sb, sim_ps)

    max8 = small.tile([BF, 8], f32)
    nc.vector.max(max8, sim_sb)

    # ordered one-hot masks, transposed into [SIZE, RK] layout (col = r*8+k)
    maskT_all = small.tile([SIZE, RK], f32)
    maskT_v = maskT_all.rearrange("s (r k) -> s r k", k=K)
    for k in range(K):
        mk = scratch.tile([BF, SIZE], f32, tag="mk", bufs=9)
        nc.vector.tensor_scalar(mk, sim_sb, max8[:, k:k + 1], None,
                                op0=alu.is_equal)
        tp = psS.tile([128, SIZE], f32, tag="tp")
        nc.tensor.transpose(tp[:SIZE], mk, id64)
        nc.vector.tensor_copy(maskT_v[:, :, k], tp[:SIZE])

    # recon / diff loss rows
    maska = scratch.tile([BF, SIZE], f32, tag="mk", bufs=9)
    nc.vector.tensor_scalar(maska, sim_sb, max8[:, 7:8], None, op0=alu.is_ge)
    simM = scratch.tile([BF, SIZE], f32, tag="kkd")
    nc.vector.tensor_tensor(simM, sim_sb, maska, op=alu.mult)
    tpm = psS.tile([128, SIZE], f32, tag="tp")
    nc.tensor.transpose(tpm[:SIZE], simM, id64)
    smT = small.tile([SIZE, BF], f32)
    nc.vector.tensor_copy(smT, tpm[:SIZE])
    recon_ps = psS.tile([BF, PD], f32, tag="mm")
    nc.tensor.matmul(recon_ps, smT, keys_sb, start=True, stop=True)
    dd = scratch.tile([BF, PD], f32, tag="sq")
    nc.vector.tensor_sub(dd, recon_ps, q_sb)
    dn_sb = small.tile([BF, 1], f32)
    dsq = scratch.tile([BF, PD], f32, tag="sq2")
    nc.scalar.activation(dsq, dd, act.Square, accum_out=dn_sb)
    nc.sync.dma_start(dn_out, dn_sb)

    # final gather+projection: out[rk, e] = sum_s maskT[s, rk] * KW[s, e]
    for m in range(4):
        o_ps = psA.tile([128, ED], f32, tag="acc")
        for n0, nw in ((0, 512), (512, 256)):
            nc.tensor.matmul(o_ps[:, n0:n0 + nw],
                             maskT_all[:, m * 128:(m + 1) * 128],
                             kw_sb[:, n0:n0 + nw], start=True, stop=True)
        o_sb = opool.tile([128, ED], f32, tag="o")
        nc.vector.tensor_copy(o_sb, o_ps)
        nc.sync.dma_start(out_mat[m * 128:(m + 1) * 128], o_sb)


def _build():
    global _BUILT
    if _BUILT is not None:
        return _BUILT
    from contextlib import ExitStack
    import concourse.bass as bass
    import concourse.bacc as bacc
    import concourse.mybir as mybir
    import concourse.tile as tile

    f32 = mybir.dt.float32
    f16 = mybir.dt.float16
    nc = bacc.Bacc("TRN2", target_bir_lowering=False, debug=False,
                   enable_asserts=False, enable_partition_id=False)

    xb = nc.dram_tensor("xb", [TP, 2, ED], f16, kind="ExternalInput").ap()
    ssel = nc.dram_tensor("ssel", [TP_PAD, BF], f16, kind="ExternalInput").ap()
    winT = nc.dram_tensor("winT", [ED, PD], f32, kind="ExternalInput").ap()
    woutT = nc.dram_tensor("woutT", [PD, ED], f32, kind="ExternalInput").ap()
    pv = nc.dram_tensor("pv", [SIZE, PD], f32, kind="ExternalInput").ap()
    ident = nc.dram_tensor("ident", [128, 128], f32, kind="ExternalInput").ap()
    out_mat = nc.dram_tensor("out_mat", [RK, ED], f32, kind="ExternalOutput").ap()
    dn_out = nc.dram_tensor("dn", [BF, 1], f32, kind="ExternalOutput").ap()
    kk_out = nc.dram_tensor("kkrow", [SIZE, 1], f32, kind="ExternalOutput").ap()

    aps = (xb, ssel, winT, woutT, pv, ident, out_mat, dn_out, kk_out)
    with tile.TileContext(nc) as tc:
        with ExitStack() as ctx:
            _build_body(tc, ctx, aps)
    nc.compile()
    _BUILT = nc
    return nc


def make_in_maps(x, W_in, W_out, prompt_values):
    x = np.asarray(x, dtype=np.float32)
    W_in = np.asarray(W_in, dtype=np.float32)
    W_out = np.asarray(W_out, dtype=np.float32)
    pv = np.asarray(prompt_values, dtype=np.float32)[:, 0, :].copy()

    fid = np.arange(TP) // NT
    S = np.zeros((TP_PAD, BF), np.float16)
    S[np.arange(TP), fid] = 1.0

    base = {
        "ssel": S,
        "winT": np.ascontiguousarray(W_in.T),
        "woutT": np.ascontiguousarray(W_out.T),
        "pv": pv,
        "ident": np.eye(128, dtype=np.float32),
    }
    in_maps = []
    for c in range(NCORES):
        sh = x[c * BP:(c + 1) * BP].reshape(TP, ED)
        xb = np.empty((TP, 2, ED), np.float16)
        hi = sh.astype(np.float16)
        xb[:, 0, :] = hi
        xb[:, 1, :] = (sh - hi.astype(np.float32)).astype(np.float16)
        in_maps.append(dict(base, xb=xb))
    return in_maps


def postprocess(results):
    out = np.concatenate(
        [r["out_mat"].reshape(BP, F * K, ED) for r in results], axis=0)
    diff = sum(float(r["dn"].sum()) for r in results) / float(B * F)
    ksim = float(results[0]["kkrow"].sum()) / float(B * F)
    ps_loss = np.array([(diff + ksim) * F], dtype=np.float32)
    return out, ps_loss


def kernel(x, W_in, W_out, prompt_values):
    from concourse import bass_utils
    nc = _build()
    in_maps = make_in_maps(x, W_in, W_out, prompt_values)
    res = bass_utils.run_bass_kernel_spmd(nc, in_maps,
                                          core_ids=list(range(NCORES)))
    return postprocess(res.results)
